# revision 33
# baseline (speedup 1.0000x reference)
"""Trainium2 Bass kernel for Mistral-style GQA attention (8-core head-parallel).

Sharding: tensor-parallel over heads. Each of the 8 cores owns 4 query
heads + their shared KV head (GQA group), computes q/k/v projections,
RoPE, causal attention and its slice of the o_proj contraction; the host
sums the 8 partial outputs (the all-reduce of the sharding hint).

v1 design (fused single pipeline, all-bf16 matmuls):
  - One pass per 512-token query tile: project qkv (per-output psum
    accumulation chains), RoPE epilogue, causal attention for that tile,
    o_proj. q/k/v stay SBUF-resident (no DRAM roundtrip).
  - All matmul operands bf16: same PE rate as fp32r (1 col/cycle) but
    half the DMA/SBUF footprint and FWL-accelerated weight loads.
  - Softmax denominators: the 4 heads' ones-matmuls are packed into the
    four 32-column groups of the PE array (tile_position), so they cost
    one matmul slot instead of four. All 4 land in one PSUM bank at
    partition rows 0/32/64/96.
  - Normalization: reciprocal rows are written into per-head zero-masked
    [128, TT] tiles; a full-K ones matmul broadcasts row 32h to all 128
    partitions (no cross-partition copies needed).
  - o_proj of tile t-1 is interleaved chunk-by-chunk into tile t's
    attention jb loop: the PE fills the latency of the scalar engine's
    exp stream (ACT is the attention-phase bottleneck at ~720ns per
    [128,512] exp vs 213ns per matmul).
  - Row max for softmax is replaced by a constant shift (scores bounded,
    exp(s-25) neither overflows nor fully underflows).
"""

import numpy as np

import concourse.bass as bass
import concourse.tile as tile
from concourse import mybir
from concourse.bass_utils import run_bass_kernel_spmd
from concourse.masks import make_identity

F32 = mybir.dt.float32
F32R = mybir.dt.float32r
BF16 = mybir.dt.bfloat16
N_CORES = 8
D = 128          # head dim
QH = 4           # query heads per core
QF = QH * D      # 512 local q features
EXP_SHIFT = 25.0
NEG = -1.0e30

CFG_FULL = dict(B=2, S=2048, H=4096)


def r(ap):
    return ap.bitcast(F32R)


# ---------------------------------------------------------------- program

def build_program(cfg):
    B, S, H = cfg["B"], cfg["S"], cfg["H"]
    T = B * S
    HC = H // 128          # contraction chunks for projections
    TT = 512               # token tile
    NTB = S // TT          # tiles per batch
    EXPFN = mybir.ActivationFunctionType.Exp
    COPYFN = mybir.ActivationFunctionType.Copy

    nc = bass.Bass("TRN2", target_bir_lowering=False, debug=False,
                   num_devices=N_CORES)

    xT = nc.dram_tensor("xT", [H, T], BF16, kind="ExternalInput").ap()
    # weights host-packed partition-major: w_r[p, hc*F + f] = w.T[hc*128+p, f]
    wqT = nc.dram_tensor("wqT", [128, HC * QF], BF16,
                         kind="ExternalInput").ap()
    wkT = nc.dram_tensor("wkT", [128, HC * D], BF16,
                         kind="ExternalInput").ap()
    wvT = nc.dram_tensor("wvT", [128, HC * D], BF16,
                         kind="ExternalInput").ap()
    woT = nc.dram_tensor("woT", [QF, H], BF16, kind="ExternalInput").ap()
    cosk = nc.dram_tensor("cosk", [D, S], F32, kind="ExternalInput").ap()
    sink = nc.dram_tensor("sink", [D, S], F32, kind="ExternalInput").ap()
    tri = nc.dram_tensor("tri", [128, 128], F32, kind="ExternalInput").ap()
    opart = nc.dram_tensor("opart", [T, H], BF16, kind="ExternalOutput").ap()

    with tile.TileContext(nc) as tc:
        if True:
            consts = tc.alloc_tile_pool(name="consts", bufs=1)
            wq_pool = tc.alloc_tile_pool(name="wq", bufs=1)
            wk_pool = tc.alloc_tile_pool(name="wk", bufs=1)
            wv_pool = tc.alloc_tile_pool(name="wv", bufs=1)
            wo_pool = tc.alloc_tile_pool(name="wo", bufs=QH)
            x_pool = tc.alloc_tile_pool(name="x", bufs=50)
            cs_pool = tc.alloc_tile_pool(name="cs", bufs=2)
            ep_pool = tc.alloc_tile_pool(name="ep", bufs=2)
            q_pool = tc.alloc_tile_pool(name="qsb", bufs=8)
            kv_pool = tc.alloc_tile_pool(name="kv", bufs=1)
            vT_pool = tc.alloc_tile_pool(name="vt", bufs=2)
            pexp_pool = tc.alloc_tile_pool(name="pexp", bufs=8)
            at_pool = tc.alloc_tile_pool(name="at", bufs=8)
            bc_pool = tc.alloc_tile_pool(name="bc", bufs=2)
            osb_pool = tc.alloc_tile_pool(name="osb", bufs=2)
            ps_pool = tc.alloc_tile_pool(name="ps", bufs=4, space="PSUM")

            # ---------------- weights + constants (first-tile x interleaved)
            # big contiguous weight DMAs; wq split in 4 so the first
            # projection chain can start streaming early; cos/sin slices for
            # tile 0 land before the first rope epilogue needs them
            wq_sb = wq_pool.tile([128, HC * QF], BF16, tag="wq", bufs=1)
            wk_sb = wk_pool.tile([128, HC * D], BF16, tag="wk", bufs=1)
            wv_sb = wv_pool.tile([128, HC * D], BF16, tag="wv", bufs=1)
            tri_sb = consts.tile([128, 128], F32)
            x_cur = []

            def load_cs(t):
                ct_ = cs_pool.tile([128, TT], F32, tag="cos", name=f"cos{t}")
                nc.sync.dma_start(ct_[:], cosk[:, t * TT:(t + 1) * TT])
                st_ = cs_pool.tile([128, TT], F32, tag="sin", name=f"sin{t}")
                nc.sync.dma_start(st_[:], sink[:, t * TT:(t + 1) * TT])
                return ct_, st_

            # wq is host-packed BY HEAD: chain h only needs its 1 MB quarter,
            # so the first chains pace with the x stream instead of the
            # full 4 MB; everything not needed until later loads after.
            HB = HC * 128  # columns per head block in wq_sb
            for h in range(QH):
                nc.sync.dma_start(
                    wq_sb[:, h * HB:(h + 1) * HB], wqT[:, h * HB:(h + 1) * HB])
                if h < 2:
                    for hc in range(h * 16, h * 16 + 16):
                        xx = x_pool.tile([128, TT], BF16, tag="x",
                                         name=f"x0_{hc}")
                        nc.sync.dma_start(
                            xx[:], xT[hc * 128:(hc + 1) * 128, 0:TT])
                        x_cur.append(xx)
            cs_cur = load_cs(0)
            nc.sync.dma_start(wk_sb[:], wkT[:])
            nc.sync.dma_start(wv_sb[:], wvT[:])
            nc.sync.dma_start(tri_sb[:], tri[:])
            ident = consts.tile([128, 128], BF16)
            make_identity(nc, ident[:])
            ones_bf = consts.tile([128, 128], BF16)
            nc.vector.memset(ones_bf[:], 1.0)
            neg_shift = consts.tile([128, 1], F32)
            nc.vector.memset(neg_shift[:], -EXP_SHIFT)
            # pre-warm the ACT exp table set off the critical path
            scratch1 = consts.tile([128, 1], F32)
            nc.scalar.activation(scratch1[:], neg_shift[:], EXPFN)
            rmask = []
            for h in range(QH):
                rm = consts.tile([128, TT], BF16, tag=f"rm{h}",
                                 name=f"rmask{h}")
                nc.vector.memset(rm[:], 0.0)
                rmask.append(rm)

            wo_t = []
            for h in range(QH):
                w = wo_pool.tile([128, H], BF16, tag="wo", name=f"wo{h}")
                nc.sync.dma_start(w[:], woT[h * 128:(h + 1) * 128, :])
                wo_t.append(w)

            # persistent per-batch k/v (rewritten each batch; Tile handles WAR)
            kT_sb = kv_pool.tile([128, S], BF16, tag="kt")
            v_big = kv_pool.tile([128, S], BF16, tag="vb")

            # ---------------- helpers
            def rope_store(ps, dst, cs):
                """dst[:, :] = rope(ps) with (cos, sin) tiles cs."""
                ct = ep_pool.tile([128, TT], F32, tag="ct")
                nc.vector.tensor_mul(ct[:], ps[:], cs[0][:])
                st_ = ep_pool.tile([128, TT], F32, tag="st")
                nc.vector.tensor_mul(st_[:], ps[:], cs[1][:])
                sr = ep_pool.tile([128, TT], F32, tag="sr")
                nc.gpsimd.dma_start(sr[0:64, :], st_[64:128, :])
                nc.gpsimd.dma_start(sr[64:128, :], st_[0:64, :])
                nc.vector.tensor_add(dst, ct[:], sr[:])

            def make_o_chunks(ats, r0):
                """o_proj emitters for one tile: 16 chunks (4 st x 4 mt-pairs)
                -> list of closures, each emitting 8 matmuls + 2 copies."""
                chunks = []
                osb_box = {}

                def emit(st, mt):
                    if mt % 4 == 0:
                        osb_box[st] = osb_pool.tile([128, H // 2], BF16,
                                                    tag="osb",
                                                    name=f"osb{st}_{mt}")
                    osb = osb_box[st]
                    ps_o = ps_pool.tile([128, 512], F32, tag="shared",
                                        name=f"pso{st}_{mt}")
                    for h2 in range(QH):
                        nc.tensor.matmul(
                            ps_o[:],
                            ats[h2][:, st * 128:(st + 1) * 128],
                            wo_t[h2][:, mt * 512:(mt + 1) * 512],
                            start=(h2 == 0), stop=(h2 == QH - 1))
                    if mt % 2 == 0:
                        nc.vector.tensor_copy(
                            osb[:, (mt % 4) * 512:(mt % 4 + 1) * 512], ps_o[:])
                    else:
                        nc.scalar.activation(
                            osb[:, (mt % 4) * 512:(mt % 4 + 1) * 512],
                            ps_o[:], COPYFN)
                    if mt % 2 == 1:
                        nc.gpsimd.dma_start(
                            opart[r0 + st * 128:r0 + (st + 1) * 128,
                                  (mt - 1) * 512:(mt + 1) * 512],
                            osb[:, (mt % 4 - 1) * 512:(mt % 4 + 1) * 512])

                for st in range(4):
                    for mt in range(8):
                        chunks.append(lambda st=st, mt=mt: emit(st, mt))
                return chunks

            # ---------------- fused main loop
            pending = []           # o_proj chunks of the previous tile

            def pop_pending(k):
                for _ in range(min(k, len(pending))):
                    pending.pop(0)()

            for b in range(B):
                for t in range(NTB):
                    r0 = b * S + t * TT
                    njb = (t + 1) * (TT // 128)

                    # x loads for this tile (first tile preloaded above)
                    if r0 != 0:
                        x_cur = []
                        for hc in range(HC):
                            xx = x_pool.tile([128, TT], BF16, tag="x",
                                             name=f"x{r0}_{hc}")
                            nc.sync.dma_start(
                                xx[:], xT[hc * 128:(hc + 1) * 128, r0:r0 + TT])
                            x_cur.append(xx)
                        cs_cur = load_cs(t)

                    # number of pending o-chunks to emit per slot
                    spare = max(0, len(pending) - njb - 4)
                    per_chain = -(-spare // 6) if spare else 0

                    # ---- projection chains: q0..q3, k, v
                    q_sb = []
                    for h in range(QH):
                        ps = ps_pool.tile([128, TT], F32, tag="shared",
                                          name=f"psq{h}", bufs=4)
                        for hc in range(HC):
                            nc.tensor.matmul(
                                ps[:],
                                wq_sb[:, (h * HC + hc) * 128:
                                      (h * HC + hc + 1) * 128],
                                x_cur[hc][:], start=(hc == 0),
                                stop=(hc == HC - 1))
                        qt = q_pool.tile([128, TT], BF16, tag="q",
                                         name=f"q{h}")
                        rope_store(ps, qt[:], cs_cur)
                        q_sb.append(qt)
                        pop_pending(per_chain)

                    ps_k = ps_pool.tile([128, TT], F32, tag="shared",
                                        name="psk", bufs=4)
                    for hc in range(HC):
                        nc.tensor.matmul(
                            ps_k[:], wk_sb[:, hc * D:(hc + 1) * D],
                            x_cur[hc][:],
                            start=(hc == 0), stop=(hc == HC - 1))
                    rope_store(ps_k, kT_sb[:, t * TT:(t + 1) * TT], cs_cur)
                    pop_pending(per_chain)

                    ps_v = ps_pool.tile([128, TT], F32, tag="shared",
                                        name="psv", bufs=4)
                    for hc in range(HC):
                        nc.tensor.matmul(
                            ps_v[:], wv_sb[:, hc * D:(hc + 1) * D],
                            x_cur[hc][:],
                            start=(hc == 0), stop=(hc == HC - 1))
                    vT = vT_pool.tile([128, TT], BF16, tag="vT")
                    nc.vector.tensor_copy(vT[:], ps_v[:])
                    ps_vt = ps_pool.tile([128, TT], BF16, tag="shared",
                                         name="psvt", bufs=4)
                    for k2 in range(TT // 128):
                        nc.tensor.transpose(
                            ps_vt[:, k2 * 128:(k2 + 1) * 128],
                            vT[:, k2 * 128:(k2 + 1) * 128], ident[:])
                    nc.vector.tensor_copy(
                        v_big[:, t * TT:(t + 1) * TT], ps_vt[:])
                    pop_pending(per_chain)

                    # ---- attention: two sweeps of head pairs (PSUM budget:
                    # each head's denominator needs its own bank at
                    # partition 0 -- nonzero-base matmul outputs mis-execute)
                    ats = [None] * QH
                    for pair in range(2):
                        heads = (2 * pair, 2 * pair + 1)
                        ps_attn = {h: ps_pool.tile([128, TT], F32, tag="attn",
                                                   name=f"psattn{h}", bufs=2)
                                   for h in heads}
                        ps_sums = {h: ps_pool.tile([128, TT], F32, tag="sums",
                                                   name=f"pssums{h}", bufs=2)
                                   for h in heads}
                        for jb in range(njb):
                            off = max(0, jb * 128 - t * TT)
                            j0 = jb * 128
                            pexps = {}
                            for h in heads:
                                ps_sc = ps_pool.tile(
                                    [128, TT], F32, tag="shared",
                                    name=f"sc{h}", bufs=4)
                                nc.tensor.matmul(
                                    ps_sc[:, off:TT],
                                    kT_sb[:, j0:j0 + 128],
                                    q_sb[h][:, off:TT],
                                    start=True, stop=True)
                                if jb >= t * (TT // 128):
                                    nc.vector.tensor_add(
                                        ps_sc[:, off:off + 128],
                                        ps_sc[:, off:off + 128], tri_sb[:])
                                pexp = pexp_pool.tile([128, TT], BF16,
                                                      tag="pe",
                                                      name=f"pexp{h}")
                                nc.scalar.activation(
                                    pexp[:, off:TT], ps_sc[:, off:TT], EXPFN,
                                    bias=neg_shift[:])
                                pexps[h] = pexp
                            # PE fill while ACT streams the exps
                            if jb % 2 == pair:
                                pop_pending(1)
                            for h in heads:
                                nc.tensor.matmul(
                                    ps_attn[h][:, off:TT],
                                    v_big[:, j0:j0 + 128],
                                    pexps[h][:, off:TT],
                                    start=(jb == 0), stop=(jb == njb - 1))
                                nc.tensor.matmul(
                                    ps_sums[h][0:1, off:TT],
                                    ones_bf[:, 0:1],
                                    pexps[h][:, off:TT],
                                    start=(jb == 0), stop=(jb == njb - 1))

                        # normalize this pair: 1/sums row -> masked bcast;
                        # o-chunk fills cover the ACT recip latency
                        for h in heads:
                            lsb = bc_pool.tile([1, TT], F32, tag="lsb",
                                               name=f"lsb{h}", bufs=2)
                            nc.scalar.activation(
                                lsb[:], ps_sums[h][0:1, :],
                                mybir.ActivationFunctionType.Ln)
                            nc.scalar.activation(
                                rmask[h][0:1, :], lsb[:], EXPFN, scale=-1.0)
                        pop_pending(2)
                        for h in heads:
                            ps_bc = ps_pool.tile([128, TT], F32, tag="shared",
                                                 name=f"psbc{h}", bufs=4)
                            nc.tensor.matmul(ps_bc[:], ones_bf[:],
                                             rmask[h][:],
                                             start=True, stop=True)
                            bc = bc_pool.tile([128, TT], BF16, tag="bc",
                                              name=f"bc{h}")
                            nc.vector.tensor_copy(bc[:], ps_bc[:])
                            at = at_pool.tile([128, TT], BF16, tag="at",
                                              name=f"at{h}")
                            nc.vector.tensor_mul(at[:], ps_attn[h][:], bc[:])
                            ats[h] = at

                    pop_pending(len(pending))  # flush any leftovers
                    pending = make_o_chunks(ats, r0)

            pop_pending(len(pending))

            for p in (ps_pool, osb_pool, bc_pool, at_pool, pexp_pool,
                      vT_pool, kv_pool, q_pool, ep_pool, cs_pool, x_pool,
                      wo_pool, wv_pool, wk_pool, wq_pool, consts):
                p.release()

    _split_multi_waits(nc)
    return nc


# ------------------------------------------------- multi-wait legalization

def _split_multi_waits(nc, cap_regular=1, cap_es=2):
    """This container's walrus enforces the HW wait-slot limits (1 sync wait
    per regular instruction, 2 per EventSemaphore); Tile can attach more.
    Engines run their stream in order, so excess waits are hoisted into
    wait-only EventSemaphore instructions immediately before the owner."""
    from bass_rust import SyncInfo

    n = 0
    for f in nc.m.functions:
        for blk in f.blocks:
            out = []
            changed = False
            for inst in blk.instructions:
                si = inst.sync_info
                waits = list(si.on_wait) if (si and si.on_wait) else []
                cap = (cap_es if isinstance(inst, mybir.InstEventSemaphore)
                       else cap_regular)
                if len(waits) > cap:
                    changed = True
                    n += 1
                    keep = waits[-cap:] if cap else []
                    extra = waits[:len(waits) - cap]
                    i = 0
                    while i < len(extra):
                        chunk = extra[i:i + cap_es]
                        es = mybir.InstEventSemaphore(
                            name=f"{inst.name}-wsplit{i}", ins=[], outs=[])
                        es.engine = inst.engine
                        es.sync_info = SyncInfo(on_wait=chunk, on_update=[])
                        out.append(es)
                        i += len(chunk)
                    inst.sync_info = SyncInfo(
                        on_wait=keep,
                        on_update=list(si.on_update) if si.on_update else [])
                out.append(inst)
            if changed:
                try:
                    blk.instructions = out
                except Exception:
                    blk.instructions.clear()
                    blk.instructions.extend(out)
    return n


# ---------------------------------------------------------------- host side

def host_prep(cfg, hidden_states, cos, sin, wq, wk, wv, wo):
    import ml_dtypes

    B, S, H = cfg["B"], cfg["S"], cfg["H"]
    T = B * S
    f32 = np.float32
    bf16 = ml_dtypes.bfloat16

    xT = np.ascontiguousarray(
        hidden_states.reshape(T, H).T).astype(bf16)
    # cos/sin identical across batch (position tables)
    cos_t = np.ascontiguousarray(cos[0].T).astype(f32, copy=False)  # [D, S]
    sign = np.concatenate([np.ones(64, f32), -np.ones(64, f32)])[:, None]
    sin_t = np.ascontiguousarray(sin[0].T * sign).astype(f32, copy=False)
    scale = np.float32(D ** -0.5)
    ii = np.arange(128)
    tri = np.where(ii[None, :] >= ii[:, None], 0.0, NEG).astype(f32)

    HC = H // 128

    def pack(wT, f):
        # [H, f] -> [128, HC*f] partition-major chunks
        return np.ascontiguousarray(
            wT.reshape(HC, 128, f).transpose(1, 0, 2).reshape(128, HC * f)
        ).astype(bf16)

    def pack_q(wT):
        # [H, QF] -> [128, QH*HC*128], by head then contraction chunk
        return np.ascontiguousarray(
            wT.reshape(HC, 128, QH, 128).transpose(1, 2, 0, 3)
            .reshape(128, QH * HC * 128)).astype(bf16)

    in_maps = []
    for c in range(N_CORES):
        in_maps.append({
            "xT": xT,
            "wqT": pack_q((wq[c * QF:(c + 1) * QF, :] * scale).T),
            "wkT": pack(wk[c * D:(c + 1) * D, :].T, D),
            "wvT": pack(wv[c * D:(c + 1) * D, :].T, D),
            "woT": np.ascontiguousarray(
                wo[:, c * QF:(c + 1) * QF].T).astype(bf16),
            "cosk": cos_t, "sink": sin_t,
            "tri": tri,
        })
    return in_maps


def assemble(cfg, results):
    B, S, H = cfg["B"], cfg["S"], cfg["H"]
    out = results[0]["opart"].astype(np.float32)
    for c in range(1, N_CORES):
        out += results[c]["opart"].astype(np.float32)
    return out.reshape(B, S, H)


def run(cfg, inputs, trace=False, **kwargs):
    nc = build_program(cfg)
    in_maps = host_prep(cfg, **{k: np.asarray(v) for k, v in inputs.items()})
    res = run_bass_kernel_spmd(nc, in_maps, core_ids=list(range(N_CORES)),
                               trace=trace, **kwargs)
    return assemble(cfg, res.results), res


def kernel(**inputs):
    # A freshly-booted device occasionally reports
    # NRT_EXEC_UNIT_UNRECOVERABLE on the first large launch; a retry on a
    # clean session has always succeeded.
    last = None
    for _ in range(3):
        try:
            out, _ = run(CFG_FULL, inputs, trace=False)
            return out
        except Exception as e:  # noqa: BLE001
            last = e
    raise last


# revision 35
# speedup vs baseline: 1.1964x; 1.1964x over previous
"""Trainium2 Bass kernel for Mistral-style GQA attention (8-core head-parallel).

Sharding: tensor-parallel over heads. Each of the 8 cores owns 4 query
heads + their shared KV head (GQA group), computes q/k/v projections,
RoPE, causal attention and its slice of the o_proj contraction; the host
sums the 8 partial outputs (the all-reduce of the sharding hint).

v1 design (fused single pipeline, all-bf16 matmuls):
  - One pass per 512-token query tile: project qkv (per-output psum
    accumulation chains), RoPE epilogue, causal attention for that tile,
    o_proj. q/k/v stay SBUF-resident (no DRAM roundtrip).
  - All matmul operands bf16: same PE rate as fp32r (1 col/cycle) but
    half the DMA/SBUF footprint and FWL-accelerated weight loads.
  - Softmax denominators: the 4 heads' ones-matmuls are packed into the
    four 32-column groups of the PE array (tile_position), so they cost
    one matmul slot instead of four. All 4 land in one PSUM bank at
    partition rows 0/32/64/96.
  - Normalization: reciprocal rows are written into per-head zero-masked
    [128, TT] tiles; a full-K ones matmul broadcasts row 32h to all 128
    partitions (no cross-partition copies needed).
  - o_proj of tile t-1 is interleaved chunk-by-chunk into tile t's
    attention jb loop: the PE fills the latency of the scalar engine's
    exp stream (ACT is the attention-phase bottleneck at ~720ns per
    [128,512] exp vs 213ns per matmul).
  - Row max for softmax is replaced by a constant shift (scores bounded,
    exp(s-25) neither overflows nor fully underflows).
"""

import numpy as np

import concourse.bass as bass
import concourse.tile as tile
from concourse import mybir
from concourse.bass_utils import run_bass_kernel_spmd
from concourse.masks import make_identity

F32 = mybir.dt.float32
F32R = mybir.dt.float32r
BF16 = mybir.dt.bfloat16
N_CORES = 8
D = 128          # head dim
QH = 4           # query heads per core
QF = QH * D      # 512 local q features
EXP_SHIFT = 25.0
NEG = -1.0e30

CFG_FULL = dict(B=2, S=2048, H=4096)


def r(ap):
    return ap.bitcast(F32R)


# ---------------------------------------------------------------- program

def build_program(cfg):
    B, S, H = cfg["B"], cfg["S"], cfg["H"]
    T = B * S
    HC = H // 128          # contraction chunks for projections
    TT = 512               # token tile
    NTB = S // TT          # tiles per batch
    EXPFN = mybir.ActivationFunctionType.Exp
    COPYFN = mybir.ActivationFunctionType.Copy

    nc = bass.Bass("TRN2", target_bir_lowering=False, debug=False,
                   num_devices=N_CORES)

    xT = nc.dram_tensor("xT", [H, T], BF16, kind="ExternalInput").ap()
    # weights host-packed partition-major: w_r[p, hc*F + f] = w.T[hc*128+p, f]
    wqT = nc.dram_tensor("wqT", [128, HC * QF], BF16,
                         kind="ExternalInput").ap()
    wkT = nc.dram_tensor("wkT", [128, HC * D], BF16,
                         kind="ExternalInput").ap()
    wvT = nc.dram_tensor("wvT", [128, HC * D], BF16,
                         kind="ExternalInput").ap()
    woT = nc.dram_tensor("woT", [QF, H], BF16, kind="ExternalInput").ap()
    cosk = nc.dram_tensor("cosk", [D, S], F32, kind="ExternalInput").ap()
    sink = nc.dram_tensor("sink", [D, S], F32, kind="ExternalInput").ap()
    tri = nc.dram_tensor("tri", [128, 128], F32, kind="ExternalInput").ap()
    opart = nc.dram_tensor("opart", [T, H], BF16, kind="ExternalOutput").ap()

    with tile.TileContext(nc) as tc:
        if True:
            consts = tc.alloc_tile_pool(name="consts", bufs=1)
            wq_pool = tc.alloc_tile_pool(name="wq", bufs=1)
            wk_pool = tc.alloc_tile_pool(name="wk", bufs=1)
            wv_pool = tc.alloc_tile_pool(name="wv", bufs=1)
            wo_pool = tc.alloc_tile_pool(name="wo", bufs=QH)
            x_pool = tc.alloc_tile_pool(name="x", bufs=50)
            cs_pool = tc.alloc_tile_pool(name="cs", bufs=2)
            ep_pool = tc.alloc_tile_pool(name="ep", bufs=2)
            q_pool = tc.alloc_tile_pool(name="qsb", bufs=8)
            kv_pool = tc.alloc_tile_pool(name="kv", bufs=1)
            vT_pool = tc.alloc_tile_pool(name="vt", bufs=2)
            pexp_pool = tc.alloc_tile_pool(name="pexp", bufs=8)
            at_pool = tc.alloc_tile_pool(name="at", bufs=8)
            bc_pool = tc.alloc_tile_pool(name="bc", bufs=2)
            osb_pool = tc.alloc_tile_pool(name="osb", bufs=2)
            ps_pool = tc.alloc_tile_pool(name="ps", bufs=4, space="PSUM")

            # ---------------- weights + constants (first-tile x interleaved)
            # big contiguous weight DMAs; wq split in 4 so the first
            # projection chain can start streaming early; cos/sin slices for
            # tile 0 land before the first rope epilogue needs them
            wq_sb = wq_pool.tile([128, HC * QF], BF16, tag="wq", bufs=1)
            wk_sb = wk_pool.tile([128, HC * D], BF16, tag="wk", bufs=1)
            wv_sb = wv_pool.tile([128, HC * D], BF16, tag="wv", bufs=1)
            tri_sb = consts.tile([128, 128], F32)
            x_cur = []

            def load_cs(t):
                ct_ = cs_pool.tile([128, TT], F32, tag="cos", name=f"cos{t}")
                nc.sync.dma_start(ct_[:], cosk[:, t * TT:(t + 1) * TT])
                st_ = cs_pool.tile([128, TT], F32, tag="sin", name=f"sin{t}")
                nc.sync.dma_start(st_[:], sink[:, t * TT:(t + 1) * TT])
                return ct_, st_

            # wq is host-packed BY HEAD: chain h only needs its 1 MB quarter,
            # so the first chains pace with the x stream instead of the
            # full 4 MB; everything not needed until later loads after.
            HB = HC * 128  # columns per head block in wq_sb
            for h in range(QH):
                nc.sync.dma_start(
                    wq_sb[:, h * HB:(h + 1) * HB], wqT[:, h * HB:(h + 1) * HB])
                if h < 2:
                    for hc in range(h * 16, h * 16 + 16):
                        xx = x_pool.tile([128, TT], BF16, tag="x",
                                         name=f"x0_{hc}")
                        nc.sync.dma_start(
                            xx[:], xT[hc * 128:(hc + 1) * 128, 0:TT])
                        x_cur.append(xx)
            cs_cur = load_cs(0)
            nc.sync.dma_start(wk_sb[:], wkT[:])
            nc.sync.dma_start(wv_sb[:], wvT[:])
            nc.sync.dma_start(tri_sb[:], tri[:])
            ident = consts.tile([128, 128], BF16)
            make_identity(nc, ident[:])
            ones_bf = consts.tile([128, 128], BF16)
            nc.vector.memset(ones_bf[:], 1.0)
            neg_shift = consts.tile([128, 1], F32)
            nc.vector.memset(neg_shift[:], -EXP_SHIFT)
            # pre-warm the ACT exp table set off the critical path
            scratch1 = consts.tile([128, 1], F32)
            nc.scalar.activation(scratch1[:], neg_shift[:], EXPFN)
            rmask = []
            for h in range(QH):
                rm = consts.tile([128, TT], BF16, tag=f"rm{h}",
                                 name=f"rmask{h}")
                nc.vector.memset(rm[:], 0.0)
                rmask.append(rm)

            wo_t = []
            for h in range(QH):
                w = wo_pool.tile([128, H], BF16, tag="wo", name=f"wo{h}")
                nc.sync.dma_start(w[:], woT[h * 128:(h + 1) * 128, :])
                wo_t.append(w)

            # persistent per-batch k/v (rewritten each batch; Tile handles WAR)
            kT_sb = kv_pool.tile([128, S], BF16, tag="kt")
            v_big = kv_pool.tile([128, S], BF16, tag="vb")

            # ---------------- helpers
            def rope_store(ps, dst, cs):
                """dst[:, :] = rope(ps) with (cos, sin) tiles cs."""
                ct = ep_pool.tile([128, TT], F32, tag="ct")
                nc.vector.tensor_mul(ct[:], ps[:], cs[0][:])
                st_ = ep_pool.tile([128, TT], F32, tag="st")
                nc.vector.tensor_mul(st_[:], ps[:], cs[1][:])
                sr = ep_pool.tile([128, TT], F32, tag="sr")
                nc.sync.dma_start(sr[0:64, :], st_[64:128, :])
                nc.sync.dma_start(sr[64:128, :], st_[0:64, :])
                nc.vector.tensor_add(dst, ct[:], sr[:])

            def make_o_chunks(ats, r0):
                """o_proj emitters for one tile: 16 chunks (4 st x 4 mt-pairs)
                -> list of closures, each emitting 8 matmuls + 2 copies."""
                chunks = []
                osb_box = {}

                def emit(st, mt):
                    if mt % 4 == 0:
                        osb_box[st] = osb_pool.tile([128, H // 2], BF16,
                                                    tag="osb",
                                                    name=f"osb{st}_{mt}")
                    osb = osb_box[st]
                    ps_o = ps_pool.tile([128, 512], F32, tag="shared",
                                        name=f"pso{st}_{mt}")
                    for h2 in range(QH):
                        nc.tensor.matmul(
                            ps_o[:],
                            ats[h2][:, st * 128:(st + 1) * 128],
                            wo_t[h2][:, mt * 512:(mt + 1) * 512],
                            start=(h2 == 0), stop=(h2 == QH - 1))
                    if mt % 2 == 0:
                        nc.vector.tensor_copy(
                            osb[:, (mt % 4) * 512:(mt % 4 + 1) * 512], ps_o[:])
                    else:
                        nc.scalar.activation(
                            osb[:, (mt % 4) * 512:(mt % 4 + 1) * 512],
                            ps_o[:], COPYFN)
                    if mt % 4 == 3:
                        nc.gpsimd.dma_start(
                            opart[r0 + st * 128:r0 + (st + 1) * 128,
                                  (mt // 4) * 2048:(mt // 4 + 1) * 2048],
                            osb[:])

                for st in range(4):
                    for mt in range(8):
                        chunks.append(lambda st=st, mt=mt: emit(st, mt))
                return chunks

            # ---------------- fused main loop
            pending = []           # o_proj chunks of the previous tile

            def pop_pending(k):
                for _ in range(min(k, len(pending))):
                    pending.pop(0)()

            for b in range(B):
                for t in range(NTB):
                    r0 = b * S + t * TT
                    njb = (t + 1) * (TT // 128)

                    # x loads for this tile (first tile preloaded above)
                    if r0 != 0:
                        x_cur = []
                        for hc in range(HC):
                            xx = x_pool.tile([128, TT], BF16, tag="x",
                                             name=f"x{r0}_{hc}")
                            nc.sync.dma_start(
                                xx[:], xT[hc * 128:(hc + 1) * 128, r0:r0 + TT])
                            x_cur.append(xx)
                        cs_cur = load_cs(t)

                    # number of pending o-chunks to emit per slot
                    spare = max(0, len(pending) - njb - 4)
                    per_chain = -(-spare // 6) if spare else 0

                    # ---- projection chains: q0..q3, k, v
                    q_sb = []
                    for h in range(QH):
                        ps = ps_pool.tile([128, TT], F32, tag="shared",
                                          name=f"psq{h}", bufs=4)
                        for hc in range(HC):
                            nc.tensor.matmul(
                                ps[:],
                                wq_sb[:, (h * HC + hc) * 128:
                                      (h * HC + hc + 1) * 128],
                                x_cur[hc][:], start=(hc == 0),
                                stop=(hc == HC - 1))
                        qt = q_pool.tile([128, TT], BF16, tag="q",
                                         name=f"q{h}")
                        rope_store(ps, qt[:], cs_cur)
                        q_sb.append(qt)
                        pop_pending(per_chain)

                    ps_k = ps_pool.tile([128, TT], F32, tag="shared",
                                        name="psk", bufs=4)
                    for hc in range(HC):
                        nc.tensor.matmul(
                            ps_k[:], wk_sb[:, hc * D:(hc + 1) * D],
                            x_cur[hc][:],
                            start=(hc == 0), stop=(hc == HC - 1))
                    rope_store(ps_k, kT_sb[:, t * TT:(t + 1) * TT], cs_cur)
                    pop_pending(per_chain)

                    ps_v = ps_pool.tile([128, TT], F32, tag="shared",
                                        name="psv", bufs=4)
                    for hc in range(HC):
                        nc.tensor.matmul(
                            ps_v[:], wv_sb[:, hc * D:(hc + 1) * D],
                            x_cur[hc][:],
                            start=(hc == 0), stop=(hc == HC - 1))
                    vT = vT_pool.tile([128, TT], BF16, tag="vT")
                    nc.vector.tensor_copy(vT[:], ps_v[:])
                    ps_vt = ps_pool.tile([128, TT], BF16, tag="shared",
                                         name="psvt", bufs=4)
                    for k2 in range(TT // 128):
                        nc.tensor.transpose(
                            ps_vt[:, k2 * 128:(k2 + 1) * 128],
                            vT[:, k2 * 128:(k2 + 1) * 128], ident[:])
                    nc.vector.tensor_copy(
                        v_big[:, t * TT:(t + 1) * TT], ps_vt[:])
                    pop_pending(per_chain)

                    # ---- attention: two sweeps of head pairs (PSUM budget:
                    # each head's denominator needs its own bank at
                    # partition 0 -- nonzero-base matmul outputs mis-execute)
                    ats = [None] * QH
                    for pair in range(2):
                        heads = (2 * pair, 2 * pair + 1)
                        ps_attn = {h: ps_pool.tile([128, TT], F32, tag="attn",
                                                   name=f"psattn{h}", bufs=2)
                                   for h in heads}
                        ps_sums = {h: ps_pool.tile([128, TT], F32, tag="sums",
                                                   name=f"pssums{h}", bufs=2)
                                   for h in heads}
                        for jb in range(njb):
                            off = max(0, jb * 128 - t * TT)
                            j0 = jb * 128
                            pexps = {}
                            for h in heads:
                                ps_sc = ps_pool.tile(
                                    [128, TT], F32, tag="shared",
                                    name=f"sc{h}", bufs=4)
                                nc.tensor.matmul(
                                    ps_sc[:, off:TT],
                                    kT_sb[:, j0:j0 + 128],
                                    q_sb[h][:, off:TT],
                                    start=True, stop=True)
                                if jb >= t * (TT // 128):
                                    nc.vector.tensor_add(
                                        ps_sc[:, off:off + 128],
                                        ps_sc[:, off:off + 128], tri_sb[:])
                                pexp = pexp_pool.tile([128, TT], BF16,
                                                      tag="pe",
                                                      name=f"pexp{h}")
                                nc.scalar.activation(
                                    pexp[:, off:TT], ps_sc[:, off:TT], EXPFN,
                                    bias=neg_shift[:])
                                pexps[h] = pexp
                            # PE fill while ACT streams the exps
                            if jb % 2 == pair:
                                pop_pending(1)
                            for h in heads:
                                nc.tensor.matmul(
                                    ps_attn[h][:, off:TT],
                                    v_big[:, j0:j0 + 128],
                                    pexps[h][:, off:TT],
                                    start=(jb == 0), stop=(jb == njb - 1))
                                nc.tensor.matmul(
                                    ps_sums[h][0:1, off:TT],
                                    ones_bf[:, 0:1],
                                    pexps[h][:, off:TT],
                                    start=(jb == 0), stop=(jb == njb - 1))

                        # normalize this pair: 1/sums row -> masked bcast;
                        # o-chunk fills cover the ACT recip latency
                        for h in heads:
                            lsb = bc_pool.tile([1, TT], F32, tag="lsb",
                                               name=f"lsb{h}", bufs=2)
                            nc.scalar.activation(
                                lsb[:], ps_sums[h][0:1, :],
                                mybir.ActivationFunctionType.Ln)
                            nc.scalar.activation(
                                rmask[h][0:1, :], lsb[:], EXPFN, scale=-1.0)
                        pop_pending(2)
                        for h in heads:
                            ps_bc = ps_pool.tile([128, TT], F32, tag="shared",
                                                 name=f"psbc{h}", bufs=4)
                            nc.tensor.matmul(ps_bc[:], ones_bf[:],
                                             rmask[h][:],
                                             start=True, stop=True)
                            bc = bc_pool.tile([128, TT], BF16, tag="bc",
                                              name=f"bc{h}")
                            nc.vector.tensor_copy(bc[:], ps_bc[:])
                            at = at_pool.tile([128, TT], BF16, tag="at",
                                              name=f"at{h}")
                            nc.vector.tensor_mul(at[:], ps_attn[h][:], bc[:])
                            ats[h] = at

                    pop_pending(len(pending))  # flush any leftovers
                    pending = make_o_chunks(ats, r0)

            pop_pending(len(pending))

            for p in (ps_pool, osb_pool, bc_pool, at_pool, pexp_pool,
                      vT_pool, kv_pool, q_pool, ep_pool, cs_pool, x_pool,
                      wo_pool, wv_pool, wk_pool, wq_pool, consts):
                p.release()

    _split_multi_waits(nc)
    return nc


# ------------------------------------------------- multi-wait legalization

def _split_multi_waits(nc, cap_regular=1, cap_es=2):
    """This container's walrus enforces the HW wait-slot limits (1 sync wait
    per regular instruction, 2 per EventSemaphore); Tile can attach more.
    Engines run their stream in order, so excess waits are hoisted into
    wait-only EventSemaphore instructions immediately before the owner."""
    from bass_rust import SyncInfo

    n = 0
    for f in nc.m.functions:
        for blk in f.blocks:
            out = []
            changed = False
            for inst in blk.instructions:
                si = inst.sync_info
                waits = list(si.on_wait) if (si and si.on_wait) else []
                cap = (cap_es if isinstance(inst, mybir.InstEventSemaphore)
                       else cap_regular)
                if len(waits) > cap:
                    changed = True
                    n += 1
                    keep = waits[-cap:] if cap else []
                    extra = waits[:len(waits) - cap]
                    i = 0
                    while i < len(extra):
                        chunk = extra[i:i + cap_es]
                        es = mybir.InstEventSemaphore(
                            name=f"{inst.name}-wsplit{i}", ins=[], outs=[])
                        es.engine = inst.engine
                        es.sync_info = SyncInfo(on_wait=chunk, on_update=[])
                        out.append(es)
                        i += len(chunk)
                    inst.sync_info = SyncInfo(
                        on_wait=keep,
                        on_update=list(si.on_update) if si.on_update else [])
                out.append(inst)
            if changed:
                try:
                    blk.instructions = out
                except Exception:
                    blk.instructions.clear()
                    blk.instructions.extend(out)
    return n


# ---------------------------------------------------------------- host side

def host_prep(cfg, hidden_states, cos, sin, wq, wk, wv, wo):
    import ml_dtypes

    B, S, H = cfg["B"], cfg["S"], cfg["H"]
    T = B * S
    f32 = np.float32
    bf16 = ml_dtypes.bfloat16

    xT = np.ascontiguousarray(
        hidden_states.reshape(T, H).T).astype(bf16)
    # cos/sin identical across batch (position tables)
    cos_t = np.ascontiguousarray(cos[0].T).astype(f32, copy=False)  # [D, S]
    sign = np.concatenate([np.ones(64, f32), -np.ones(64, f32)])[:, None]
    sin_t = np.ascontiguousarray(sin[0].T * sign).astype(f32, copy=False)
    scale = np.float32(D ** -0.5)
    ii = np.arange(128)
    tri = np.where(ii[None, :] >= ii[:, None], 0.0, NEG).astype(f32)

    HC = H // 128

    def pack(wT, f):
        # [H, f] -> [128, HC*f] partition-major chunks
        return np.ascontiguousarray(
            wT.reshape(HC, 128, f).transpose(1, 0, 2).reshape(128, HC * f)
        ).astype(bf16)

    def pack_q(wT):
        # [H, QF] -> [128, QH*HC*128], by head then contraction chunk
        return np.ascontiguousarray(
            wT.reshape(HC, 128, QH, 128).transpose(1, 2, 0, 3)
            .reshape(128, QH * HC * 128)).astype(bf16)

    in_maps = []
    for c in range(N_CORES):
        in_maps.append({
            "xT": xT,
            "wqT": pack_q((wq[c * QF:(c + 1) * QF, :] * scale).T),
            "wkT": pack(wk[c * D:(c + 1) * D, :].T, D),
            "wvT": pack(wv[c * D:(c + 1) * D, :].T, D),
            "woT": np.ascontiguousarray(
                wo[:, c * QF:(c + 1) * QF].T).astype(bf16),
            "cosk": cos_t, "sink": sin_t,
            "tri": tri,
        })
    return in_maps


def assemble(cfg, results):
    B, S, H = cfg["B"], cfg["S"], cfg["H"]
    out = results[0]["opart"].astype(np.float32)
    for c in range(1, N_CORES):
        out += results[c]["opart"].astype(np.float32)
    return out.reshape(B, S, H)


def run(cfg, inputs, trace=False, **kwargs):
    nc = build_program(cfg)
    in_maps = host_prep(cfg, **{k: np.asarray(v) for k, v in inputs.items()})
    res = run_bass_kernel_spmd(nc, in_maps, core_ids=list(range(N_CORES)),
                               trace=trace, **kwargs)
    return assemble(cfg, res.results), res


def kernel(**inputs):
    # A freshly-booted device occasionally reports
    # NRT_EXEC_UNIT_UNRECOVERABLE on the first large launch; a retry on a
    # clean session has always succeeded.
    last = None
    for _ in range(3):
        try:
            out, _ = run(CFG_FULL, inputs, trace=False)
            return out
        except Exception as e:  # noqa: BLE001
            last = e
    raise last
